# revision 39
# baseline (speedup 1.0000x reference)
"""Trainium2 Bass kernel for nn_MultiHeadAttentionLayer (edge-wise MHA with
global softmax over the edge dimension).

Strategy (8 NeuronCores, data-parallel over edges):
  - Host shards E=250000 edges into 8 shards of 31250, zero-padded to 31744
    (62 chunks x 512), pre-transposed so features land on SBUF partitions,
    and cast to bf16 (PE full-rate dtype + fast weight load).
  - Phase 1 (scores): per 512-edge chunk QT = wq.T@xiT, KET = wk.T@xjT +
    wea.T@eaT (xjT resident in SBUF -- it is reused by the V matmul later),
    KE copy to SBUF with +bk+be bias alternating between the Scalar and
    GpSimd engines, P = (QT+bq)*KET (DVE), S = HsumRep.T@P (deferred by one
    chunk so the in-order PE never waits on the DVE chain).  HsumRep[f,hd] =
    (head(f)==head(hd)) yields per-head scores REPLICATED to all 16 lanes,
    so exp runs once per chunk PAIR on [128,1024] -> resident e_full bf16 +
    per-pair Z partials via ACT accum_out.
  - Z tail: reduce zparts, DMA to DRAM, AllReduce(add) of Z[128,1] triggered
    from the (drained) gpsimd queue -> minimal trigger latency.
  - AR window (Z-independent work that keeps the PE HAM clock warm):
    VT = wv.T@xjT per chunk (PSUM) and U = (VT+bv)*e_full in place on
    e_full (DVE stt, one PSUM operand).
  - After AR: 1/Z folded into wo rows (wo2 = wo * chd), then phase B:
    outT = wo2.T@U + bo -> DRAM fp16.  The PSUM->SBUF +bo copies round-robin
    over Scalar/Vector/GpSimd so no single engine serializes the drain.
  - Host gathers and transposes back to [E, 128].
"""
import os
import sys

for _p in ("/opt/trn_rl_repo", "/root/.axon_site/_ro/trn_rl_repo"):
    if os.path.isdir(_p) and _p not in sys.path:
        sys.path.append(_p)

import numpy as np
import ml_dtypes
import concourse.bacc as bacc
import concourse.tile as tile
import concourse.mybir as mybir
from concourse.bass_utils import run_bass_kernel_spmd

F32 = mybir.dt.float32
BF16 = mybir.dt.bfloat16
AF = mybir.ActivationFunctionType
ALU = mybir.AluOpType
BF = ml_dtypes.bfloat16

E_FULL = 250000
NCORES = 8
ES = E_FULL // NCORES          # 31250 edges per core
CH = 512                       # chunk size (PSUM bank width)
NCH = (ES + CH - 1) // CH      # 62 chunks
EP = NCH * CH                  # 31744 padded edges per core
D = 128
NH = 8
DK = 16
XW = 2048                      # xi/xj DMA batch width (4 chunks)
EW = 2048                      # ea DMA batch width (4 chunks)
TAILV = ES - (NCH - 1) * CH    # valid edges in the last chunk (18)
NPAIR = NCH // 2               # 31 exp pairs
PTAIL = ES - (NPAIR - 1) * 2 * CH   # valid edges in last pair (530)

_CACHE = {}


def _build():
    if "nc" in _CACHE:
        return _CACHE["nc"]

    nc = bacc.Bacc(num_devices=NCORES)

    t_xiT = nc.dram_tensor("xiT", [D, EP], BF16, kind="ExternalInput")
    t_xjT = nc.dram_tensor("xjT", [D, EP], BF16, kind="ExternalInput")
    t_eaT = nc.dram_tensor("eaT", [32, EP], BF16, kind="ExternalInput")
    t_pkb = nc.dram_tensor("pkb", [D, 768], BF16, kind="ExternalInput")
    t_pkf = nc.dram_tensor("pkf", [D, 8], F32, kind="ExternalInput")
    t_out = nc.dram_tensor("outT", [D, EP], mybir.dt.float16, kind="ExternalOutput")

    with tile.TileContext(nc) as tc:
        with (
            tc.tile_pool(name="per", bufs=1) as per,      # persistent
            tc.tile_pool(name="wk", bufs=3) as wk,        # streaming loads
            tc.tile_pool(name="mid", bufs=2) as mid,      # intermediates
            tc.tile_pool(name="dram", bufs=1, space="DRAM") as dram,
        ):
            s_pkb = per.tile([D, 768], BF16)
            nc.sync.dma_start(s_pkb[:], t_pkb[:])
            s_wq = s_pkb[:, 0:128]
            s_wk = s_pkb[:, 128:256]
            s_wv = s_pkb[:, 256:384]
            s_wo = s_pkb[:, 384:512]
            s_wea = s_pkb[0:32, 512:640]     # we
            s_hrep = s_pkb[:, 640:768]       # HsumRep [f, hd]

            s_pkf = per.tile([D, 8], F32)
            nc.sync.dma_start(s_pkf[:], t_pkf[:])
            s_bq = s_pkf[:, 0:1]
            s_bkbe = s_pkf[:, 1:2]
            s_bv = s_pkf[:, 2:3]
            s_bo = s_pkf[:, 3:4]

            s_xjf = per.tile([D, EP], BF16)      # resident xj^T (K and V)
            e_full = per.tile([D, EP], BF16)     # resident exp -> U in place
            zparts = per.tile([D, NPAIR], F32)   # per-pair Z partials
            # the pad columns of the last chunk stay zero forever: exp only
            # ever writes the valid columns, so zero them once up front
            nc.vector.memset(e_full[:, (NCH - 1) * CH + TAILV:], 0.0)

            # ---------------- phase 1: scores ----------------
            psA_ctx = tc.tile_pool(name="psA", bufs=1, space="PSUM")
            psA = psA_ctx.__enter__()
            # PE pre-warm: dummy matmuls while the first DMAs land, so HAM
            # reaches K=8/8 before the real stream starts.
            warm = per.tile([D, CH], BF16)
            nc.vector.memset(warm[:], 0.0)
            p_warm = psA.tile([D, CH], F32, tag="pq", bufs=3, name="p_warm")
            for _ in range(16):
                nc.tensor.matmul(p_warm[:], warm[:, 0:128], warm[:],
                                 start=True, stop=True)

            # prefetch the first two ea batches so the dummy collective's
            # queue processing below cannot starve early pass-A chunks
            ea_tiles = {}
            for b in range(2):
                s_ea_p = wk.tile([32, EW], BF16, tag="ea")
                nc.gpsimd.dma_start(s_ea_p[:], t_eaT[:, b * EW:(b + 1) * EW])
                ea_tiles[b] = s_ea_p

            # dummy collective at kernel start: absorbs the runtime's
            # first-collective barrier and warms the CC path so the real
            # Z AllGather (much later) sees minimal protocol latency
            d_w1 = dram.tile([D, 1], F32)
            d_w2 = dram.tile([D, NCORES], F32)
            nc.gpsimd.dma_start(d_w1[:], s_pkf[:, 7:8])
            nc.gpsimd.collective_compute(
                "AllGather", ALU.bypass,
                replica_groups=[list(range(NCORES))],
                ins=[d_w1.opt()],
                outs=[d_w2.opt()],
                cc_dim="Free",
            )

            sps = {}         # P tiles for the S matmul (deferred one pair)
            ps8 = None

            def do_s8(c):
                nonlocal ps8
                if c % 2 == 0:
                    ps8 = psA.tile([D, 2 * CH], F32, tag="ps8", bufs=1,
                                   name=f"ps8_{c}")
                nc.tensor.matmul(ps8[:, (c % 2) * CH:(c % 2) * CH + CH],
                                 s_hrep, sps.pop(c)[:], start=True, stop=True)
                if c % 2 == 1:
                    p = c // 2
                    # exp of the valid columns only; the last pair's pad
                    # stays at the memset zeros, so Z partials always come
                    # free via accum_out (no tail reduce on the Z path)
                    nv = 2 * CH if p < NPAIR - 1 else PTAIL
                    nc.scalar.activation(e_full[:, p * 2 * CH:p * 2 * CH + nv],
                                         ps8[:, :nv], AF.Exp,
                                         bias=0.0, scale=0.25,
                                         accum_out=zparts[:, p:p + 1])

            for c in range(NCH):
                sl = slice(c * CH, (c + 1) * CH)
                if c % (XW // CH) == 0:
                    xw = min(XW, EP - c * CH)
                    nc.sync.dma_start(s_xjf[:, c * CH:c * CH + xw],
                                      t_xjT[:, c * CH:c * CH + xw])
                    s_xi = wk.tile([D, XW], BF16, tag="xi")
                    nc.sync.dma_start(s_xi[:, :xw], t_xiT[:, c * CH:c * CH + xw])
                if c % (EW // CH) == 0:
                    b = c // (EW // CH)
                    if b in ea_tiles:
                        s_ea = ea_tiles.pop(b)
                    else:
                        ew = min(EW, EP - c * CH)
                        s_ea = wk.tile([32, EW], BF16, tag="ea")
                        # ea loads ride the gpsimd DMA ring: keeps the sync
                        # ring for the big xi/xj streams
                        nc.gpsimd.dma_start(s_ea[:, :ew],
                                            t_eaT[:, c * CH:c * CH + ew])
                xsl = slice((c % (XW // CH)) * CH, (c % (XW // CH)) * CH + CH)
                esl = slice((c % (EW // CH)) * CH, (c % (EW // CH)) * CH + CH)

                p_q = psA.tile([D, CH], F32, tag="pq", bufs=3)
                nc.tensor.matmul(p_q[:], s_wq, s_xi[:, xsl], start=True, stop=True)
                p_ke = psA.tile([D, CH], F32, tag="pke", bufs=3)
                nc.tensor.matmul(p_ke[:], s_wk, s_xjf[:, sl], start=True, stop=False)
                nc.tensor.matmul(p_ke[:], s_wea, s_ea[:, esl], start=False, stop=True)
                if c > 1:
                    do_s8(c - 2)

                # KE -> SBUF with bias folded into the copy; 2:1 ACT:DVE --
                # the DVE P-chain paces the S matmuls, so keep DVE light
                s_ke = mid.tile([D, CH], BF16, tag="ke", bufs=3)
                if c % 3 != 2:
                    nc.scalar.activation(s_ke[:], p_ke[:], AF.Identity,
                                         bias=s_bkbe, scale=1.0)
                else:
                    nc.vector.tensor_scalar(s_ke[:], p_ke[:], s_bkbe, None,
                                            op0=ALU.add)
                s_p = mid.tile([D, CH], BF16, tag="p", bufs=4)
                nc.vector.scalar_tensor_tensor(s_p[:], p_q[:], s_bq, s_ke[:],
                                               op0=ALU.add, op1=ALU.mult)
                sps[c] = s_p
            do_s8(NCH - 2)
            do_s8(NCH - 1)

            psA_ctx.__exit__(None, None, None)
            psB_ctx = tc.tile_pool(name="psB", bufs=1, space="PSUM")
            psB = psB_ctx.__enter__()

            # ---------------- Z tail + AllReduce trigger ----------------
            s_zl = per.tile([D, 1], F32)
            nc.vector.tensor_reduce(s_zl[:], zparts[:],
                                    axis=mybir.AxisListType.X, op=ALU.add)
            d_zin = dram.tile([D, 1], F32)
            d_zout = dram.tile([D, NCORES], F32)
            nc.gpsimd.dma_start(d_zin[:], s_zl[:])
            nc.gpsimd.collective_compute(
                "AllGather", ALU.bypass,
                replica_groups=[list(range(NCORES))],
                ins=[d_zin.opt()],
                outs=[d_zout.opt()],
                cc_dim="Free",
            )

            # ---------------- AR window: V matmuls + U = (V+bv)*e --------
            # A DVE op with a PSUM fp32 operand runs in the slowest tier
            # (~900ns/chunk), so for 2 of 3 chunks ACT copies V to SBUF bf16
            # (+bv) and the DVE multiply runs in the fast all-16-bit 2-port
            # tier; the remaining chunks go direct so neither engine stalls.
            for c in range(NCH):
                sl = slice(c * CH, (c + 1) * CH)
                p_v = psB.tile([D, CH], F32, tag="pv", bufs=3)
                nc.tensor.matmul(p_v[:], s_wv, s_xjf[:, sl], start=True, stop=True)
                if c % 3 != 2:
                    s_v = mid.tile([D, CH], BF16, tag="v", bufs=3)
                    nc.scalar.activation(s_v[:], p_v[:], AF.Identity,
                                         bias=s_bv, scale=1.0)
                    nc.vector.tensor_tensor(e_full[:, sl], s_v[:],
                                            e_full[:, sl], op=ALU.mult)
                else:
                    nc.vector.scalar_tensor_tensor(e_full[:, sl], p_v[:], s_bv,
                                                   e_full[:, sl],
                                                   op0=ALU.add, op1=ALU.mult)

            # ---------------- global Z -> wo2 ----------------
            s_zg = per.tile([D, NCORES], F32)
            nc.gpsimd.dma_start(s_zg[:], d_zout[:])
            s_zsum = per.tile([D, 1], F32)
            nc.vector.tensor_reduce(s_zsum[:], s_zg[:],
                                    axis=mybir.AxisListType.X, op=ALU.add)
            s_chd = per.tile([D, 1], F32)
            nc.vector.reciprocal(s_chd[:], s_zsum[:])
            s_wo2 = per.tile([D, D], BF16)
            nc.vector.tensor_scalar(s_wo2[:], s_wo, s_chd[:], None,
                                    op0=ALU.mult)

            # ---------------- phase B ----------------
            # per-chunk matmuls + drains (alternating ACT/DVE -- the U
            # stream is done by now) assembled into 4-chunk store batches
            s_o = None
            for c in range(NCH):
                if c % 4 == 0:
                    s_o = mid.tile([D, 4 * CH], mybir.dt.float16, tag="o",
                                   bufs=3)
                p_o = psB.tile([D, CH], F32, tag="pout", bufs=5,
                               name=f"po_{c}")
                nc.tensor.matmul(p_o[:], s_wo2[:],
                                 e_full[:, c * CH:(c + 1) * CH],
                                 start=True, stop=True)
                osl = slice((c % 4) * CH, (c % 4 + 1) * CH)
                if c % 2 == 0:
                    nc.scalar.activation(s_o[:, osl], p_o[:], AF.Identity,
                                         bias=s_bo, scale=1.0)
                else:
                    nc.vector.tensor_scalar(s_o[:, osl], p_o[:], s_bo, None,
                                            op0=ALU.add)
                if c % 4 == 3 or c == NCH - 1:
                    q0 = (c // 4) * 4
                    # alternate store batches across two DMA rings so the
                    # 8 MB output stream is not serialized on one ring
                    deng = nc.sync if (c // 4) % 2 == 0 else nc.gpsimd
                    deng.dma_start(t_out[:, q0 * CH:(c + 1) * CH],
                                   s_o[:, :(c + 1 - q0) * CH])
            psB_ctx.__exit__(None, None, None)

    nc.compile()
    _CACHE["nc"] = nc
    return nc


def _pack_constants(wq, bq, wk, bk, wv, bv, we, be, wo, bo):
    HsumRep = np.zeros((D, D), np.float32)   # [f, hd] = (head(f)==head(hd))
    for f in range(D):
        h = f // DK
        HsumRep[f, h * DK:(h + 1) * DK] = 1.0
    pkb = np.zeros((D, 768), np.float32)
    pkb[:, 0:128] = wq
    pkb[:, 128:256] = wk
    pkb[:, 256:384] = wv
    pkb[:, 384:512] = wo
    pkb[:32, 512:640] = we
    pkb[:, 640:768] = HsumRep
    pkf = np.zeros((D, 8), np.float32)
    pkf[:, 0] = bq
    pkf[:, 1] = bk + be
    pkf[:, 2] = bv
    pkf[:, 3] = bo
    return pkb.astype(BF), pkf


def _run(inputs, trace=False):
    x_i = np.asarray(inputs["x_i"], np.float32)
    x_j = np.asarray(inputs["x_j"], np.float32)
    ea = np.asarray(inputs["edge_attr"], np.float32)
    pkb, pkf = _pack_constants(
        np.asarray(inputs["wq"], np.float32), np.asarray(inputs["bq"], np.float32),
        np.asarray(inputs["wk"], np.float32), np.asarray(inputs["bk"], np.float32),
        np.asarray(inputs["wv"], np.float32), np.asarray(inputs["bv"], np.float32),
        np.asarray(inputs["we"], np.float32), np.asarray(inputs["be"], np.float32),
        np.asarray(inputs["wo"], np.float32), np.asarray(inputs["bo"], np.float32),
    )

    in_maps = []
    for c in range(NCORES):
        sl = slice(c * ES, (c + 1) * ES)
        xiT = np.zeros((D, EP), BF)
        xiT[:, :ES] = x_i[sl].T.astype(BF)
        xjT = np.zeros((D, EP), BF)
        xjT[:, :ES] = x_j[sl].T.astype(BF)
        eaT = np.zeros((32, EP), BF)
        eaT[:, :ES] = ea[sl].T.astype(BF)
        in_maps.append(dict(xiT=xiT, xjT=xjT, eaT=eaT, pkb=pkb, pkf=pkf))

    nc = _build()
    res = run_bass_kernel_spmd(nc, in_maps, list(range(NCORES)), trace=trace)

    out = np.empty((E_FULL, D), np.float32)
    for c in range(NCORES):
        sl = slice(c * ES, (c + 1) * ES)
        out[sl] = res.results[c]["outT"][:, :ES].T.astype(np.float32)
    return out, res.exec_time_ns


def kernel(**inputs) -> np.ndarray:
    return _run(inputs)[0]
